# revision 26
# baseline (speedup 1.0000x reference)
"""Multi-head causal self-attention on 8 Trainium2 NeuronCores.

Problem: x[2, 2048, 1024] @ w_attn[1024, 3072] (+b) -> split q,k,v (16 heads,
head_size 64) -> causal softmax attention -> out [2, 2048, 1024].

Sharding: 32 (batch, head) pairs across 8 cores -> each core handles one batch
and 4 consecutive heads (batch = core // 4, heads = (core % 4) * 4 + [0..3]).
No collectives; host assembles the output.

Per-core kernel, v3 (all matmul operands bf16; host pre-casts x/w):
  - q/k projection staged by 512-wide t-groups (tg0 in the prologue, tg1-3
    dribbled as PE filler through the attention stream) -> qkT[n, t] bf16.
  - v projection per 128-row t-chunk, also filler -> V[t, 4, 65] bf16 with a
    ones column (row 64 of the PV output = softmax denominator).
  - attention per (head h, 512-wide query window G): S^T[j, i] at 128-granular
    causal resolution: 4G off-diagonal [128, 512] chunk matmuls plus the four
    diagonal chunks trimmed to their visible suffix (512/384/256/128 wide),
    packed tightly into [128, 1024] PSUM tiles (pieces split at 512-column
    bank boundaries). One wide exp per tile (scale = 1/8; no max-subtraction:
    |logits|*scale < ~8); per-diagonal-chunk [128, 128] tril multiplies on
    the bf16 exp tile (DVE 2x); PV accumulates out^T[d, i] (+denominator row)
    per piece, software-pipelined two PSUM tiles behind QK so the in-order PE
    queue never blocks on an exp.
  - tail per block: PSUM->SBUF copy, 4 PE transposes into the same (just
    freed) PSUM bank, reciprocal of the denominator row, normalize into the
    output tile; outputs DMAd per 512-row group as they complete.
"""

from collections import deque

import numpy as np

import concourse.bacc as bacc
import concourse.tile as tile
from concourse import mybir

AF = mybir.ActivationFunctionType
F32 = mybir.dt.float32
BF16 = mybir.dt.bfloat16

B, T, C = 2, 2048, 1024
H, D = 16, 64
HPC = 4                 # heads per core
NCORES = 8
KC = C // 128           # 8 contraction chunks
TC = T // 128           # 16 key/time chunks of 128
NG = T // 512           # 4 query windows of 512
NQK = 2 * HPC * D       # 512 q+k columns per core
NV = HPC * D            # 256 v columns per core
DP = D + 1              # v columns + ones column
SCALE = D ** -0.5


def build_nc(reps=1):
    """reps > 1 repeats the whole compute serially inside one NEFF —
    used only for differential on-device timing (tunnel overhead cancels)."""
    nc = bacc.Bacc("TRN2", target_bir_lowering=False, debug=False,
                   num_devices=NCORES)

    xT = nc.dram_tensor("xT", [C, T], BF16, kind="ExternalInput")
    w_qk = nc.dram_tensor("w_qk", [C, NQK], BF16, kind="ExternalInput")
    w_v = nc.dram_tensor("w_v", [C, NV], BF16, kind="ExternalInput")
    b_qk = nc.dram_tensor("b_qk", [NQK, 1], F32, kind="ExternalInput")
    b_v = nc.dram_tensor("b_v", [1, NV], F32, kind="ExternalInput")
    tri = nc.dram_tensor("tri", [128, 128], BF16, kind="ExternalInput")
    ident = nc.dram_tensor("ident", [128, 128], BF16, kind="ExternalInput")
    out = nc.dram_tensor("out", [T, NV], F32, kind="ExternalOutput")

    with tile.TileContext(nc) as tc:
      for rep in range(reps):
        with (
            tc.tile_pool(name=f"const{rep}", bufs=1) as cpool,
            tc.tile_pool(name=f"xw{rep}", bufs=1) as xw,
            tc.tile_pool(name=f"qkv{rep}", bufs=1) as qkv,
            tc.tile_pool(name=f"outp{rep}", bufs=1) as outp,
            tc.tile_pool(name=f"ptp{rep}", bufs=4) as ptp,
            tc.tile_pool(name=f"otp{rep}", bufs=2) as otp,
            tc.tile_pool(name=f"recp{rep}", bufs=2) as recp,
            tc.tile_pool(name=f"warmp{rep}", bufs=1) as warmp,
        ):
            # ---------- ACT exp-table warmup (hide the first-use load) ------
            warm = warmp.tile([128, 1], F32, tag="warm")
            nc.vector.memset(warm[:], 0.0)
            nc.scalar.activation(warm[:], warm[:], AF.Exp)

            # ---------- DMAs, in consumption order, batched ----------
            wqk_sb = xw.tile([128, KC, NQK], BF16)
            wv_sb = xw.tile([128, KC, NV], BF16)
            xth = [xw.tile([128, KC, 512], BF16, tag=f"xt{tg}",
                           name=f"xt{tg}") for tg in range(NG)]
            wqk_r = w_qk[:].rearrange("(kc p) n -> p kc n", p=128)
            xT_r = xT[:].rearrange("(kc p) t -> p kc t", p=128)
            first_groups = [slice(0, 2), slice(2, 4), slice(4, 6),
                            slice(6, 8)]
            for kcs in first_groups:
                nc.sync.dma_start(wqk_sb[:, kcs, :], wqk_r[:, kcs, :])
                nc.sync.dma_start(xth[0][:, kcs, :], xT_r[:, kcs, 0:512])
            bqk_sb = cpool.tile([128, 4, 1], F32)
            nc.sync.dma_start(
                bqk_sb[:], b_qk[:].rearrange("(c p) one -> p c one", p=128)
            )
            nc.sync.dma_start(
                wv_sb[:], w_v[:].rearrange("(kc p) n -> p kc n", p=128)
            )
            bv_sb = cpool.tile([128, NV], F32)
            nc.sync.dma_start(bv_sb[:], b_v[:].to_broadcast([128, NV]))
            tri_sb = cpool.tile([128, 128], BF16)
            nc.sync.dma_start(tri_sb[:], tri[:])
            ident_sb = cpool.tile([128, 128], BF16)
            nc.sync.dma_start(ident_sb[:], ident[:])
            for tg in range(1, NG):
                nc.sync.dma_start(
                    xth[tg][:], xT_r[:, :, tg * 512:(tg + 1) * 512]
                )

            # ---------- persistent sbuf tiles ----------
            qkts = {n: qkv.tile([128, T], BF16, tag=f"qk{n}", name=f"qk{n}")
                    for n in range(4)}
            vs = [None] * TC
            outs = outp.tile([128, TC, NV], F32, tag="outs", name="outs")

            with (
                tc.tile_pool(name=f"fill{rep}", bufs=1, space="PSUM") as fillp,
                tc.tile_pool(name=f"psS{rep}", bufs=2, space="PSUM") as psS,
                tc.tile_pool(name=f"pso{rep}", bufs=3, space="PSUM") as psop,
            ):
                filler = deque()   # (level, emit_fn) items, FIFO
                pv_q = deque()     # deferred PV emitters, depth 2

                def emit_proj_group(tg, n, pool=None, tag="fill"):
                    """qkT[n-chunk, tg-group] = w_qk[:,nchunk].T @ x^T + b."""
                    pp = (pool or fillp).tile([128, 512], F32, tag=tag,
                                              name=f"pp{tg}_{n}")

                    def mk_mm(kc):
                        def emit():
                            nc.tensor.matmul(
                                pp[:],
                                wqk_sb[:, kc, n * 128:(n + 1) * 128],
                                xth[tg][:, kc, :],
                                start=(kc == 0),
                                stop=(kc == KC - 1),
                            )
                        return emit

                    def fini():
                        nc.vector.tensor_scalar_add(
                            qkts[n][:, tg * 512:(tg + 1) * 512],
                            pp[:], bqk_sb[:, n, :],
                        )

                    return [mk_mm(kc) for kc in range(KC)] + [fini]

                def emit_vproj_group(jc, pool=None, tag="fill"):
                    """vs[jc] = x^T[:, jc].T @ w_v + b_v, plus ones column."""
                    pv = (pool or fillp).tile([128, 512], F32, tag=tag,
                                              name=f"pv{jc}")
                    vt = qkv.tile([128, HPC, DP], BF16, tag=f"v{jc}",
                                  name=f"v{jc}")

                    def mk_mm(kc):
                        def emit():
                            nc.tensor.matmul(
                                pv[:, 0:NV],
                                xth[jc // 4][:, kc, (jc % 4) * 128:
                                             (jc % 4 + 1) * 128],
                                wv_sb[:, kc, :],
                                start=(kc == 0),
                                stop=(kc == KC - 1),
                            )
                        return emit

                    def fini():
                        nc.vector.tensor_tensor(
                            vt[:, :, 0:D],
                            pv[:, 0:NV].rearrange("p (h d) -> p h d", d=D),
                            bv_sb[:].rearrange("p (h d) -> p h d", d=D),
                            op=mybir.AluOpType.add,
                        )
                        nc.vector.memset(vt[:, :, D:DP], 1.0)
                        vs[jc] = vt

                    return [mk_mm(kc) for kc in range(KC)] + [fini]

                def drain_filler(n):
                    for _ in range(n):
                        if filler:
                            filler.popleft()[1]()

                def flush_level(level):
                    while filler and filler[0][0] <= level:
                        filler.popleft()[1]()

                def flush_filler():
                    while filler:
                        filler.popleft()[1]()

                def need_v(cj):
                    while vs[cj] is None:
                        assert filler, f"v{cj} not queued"
                        filler.popleft()[1]()

                def flush_pv(keep=0):
                    while len(pv_q) > keep:
                        pv_q.popleft()()

                # ---------- prologue: tg0 projection + v chunks 0..3 ----
                # borrow the (still idle) psS pool so consecutive groups
                # rotate over two banks instead of serializing on one
                for n in range(4):
                    for it in emit_proj_group(0, n, pool=psS, tag="psS"):
                        it()
                for jc in range(4):
                    for it in emit_vproj_group(jc, pool=psS, tag="psS"):
                        it()

                # ---------- filler queue: tg1..3 proj + v chunks 4..15 ----
                for lvl in (1, 2, 3):
                    for n in range(4):
                        for it in emit_proj_group(lvl, n):
                            filler.append((lvl, it))
                    for jc in range(4 * lvl, 4 * lvl + 4):
                        for it in emit_vproj_group(jc):
                            filler.append((lvl + 0.6, it))

                def emit_attn_tail(state):
                    """Transpose out^T, normalize by the denominator row.
                    Transposes reuse the pso PSUM bank just read."""
                    h, G, pso = state
                    oT = otp.tile([DP, 512], BF16, tag="oT",
                                  name=f"oT{h}_{G}")
                    nc.vector.tensor_copy(oT[:], pso[0:DP, :])
                    # 66-wide bf16 slots keep each transpose output 4B-aligned
                    pt2 = pso[:, 0:132].bitcast(BF16).rearrange(
                        "p (ic d) -> p ic d", d=DP + 1
                    )
                    for ic in range(4):
                        nc.tensor.matmul(
                            pt2[:, ic, 0:DP],
                            oT[:, ic * 128:(ic + 1) * 128],
                            ident_sb[0:DP, 0:DP],
                            is_transpose=True,
                            start=(ic == 0),
                            stop=(ic == 3),
                        )
                    rec = recp.tile([128, 4, 1], F32, tag="rec",
                                    name=f"rec{h}_{G}")
                    nc.vector.reciprocal(rec[:], pt2[:, :, D:DP])
                    for ic in range(4):
                        nc.vector.tensor_scalar_mul(
                            outs[:, 4 * G + ic, h * D:(h + 1) * D],
                            pt2[:, ic, 0:D], rec[:, ic, :],
                        )

                def row_dma(G):
                    nc.sync.dma_start(
                        out[G * 512:(G + 1) * 512, :]
                        .rearrange("(tc p) n -> p tc n", p=128),
                        outs[:, 4 * G:4 * G + 4, :],
                    )

                def emit_attn_pair(hp, G, pending, dma_G):
                    """Paired attention block: heads A=2hp (PE rows 0:64)
                    and B=2hp+1 (rows 64:128). Each pack tile holds one
                    causal piece for BOTH heads ([A @ bank0 | B @ bank1]):
                    the two QK matmuls have disjoint row-groups
                    (tile_position auto-derives from base_partition) and
                    disjoint PSUM banks, so the PE runs them concurrently.
                    `pending` tails (previous pair) are emitted after pack
                    2, before this pair's pso allocations claim their
                    banks; `dma_G` row group is flushed right after."""
                    hA, hB = 2 * hp, 2 * hp + 1
                    qTA = qkts[hp][0:D, :]
                    qTB = qkts[hp][D:128, :]
                    kTA = qkts[2 + hp][0:D, :]
                    kTB = qkts[2 + hp][D:128, :]
                    segs = [(cj, 0, False) for cj in range(4 * G)]
                    segs += [(4 * G + v, 128 * v, True) for v in range(4)]
                    nseg = len(segs)
                    hold = {}

                    def get_pso():
                        if not hold:
                            hold["A"] = psop.tile(
                                [128, 512], F32, tag="pso",
                                name=f"psoA{hp}_{G}")
                            hold["B"] = psop.tile(
                                [128, 512], F32, tag="pso",
                                name=f"psoB{hp}_{G}")
                        return hold["A"], hold["B"]

                    for idx, (cj, lo, tri) in enumerate(segs):
                        w = 512 - lo
                        pss = psS.tile([128, 1024], F32, tag="psS",
                                       name=f"pss{hp}_{G}_{idx}")
                        for kT, qT, c0 in ((kTA, qTA, 0), (kTB, qTB, 512)):
                            nc.tensor.matmul(
                                pss[:, c0:c0 + w],
                                kT[:, cj * 128:(cj + 1) * 128],
                                qT[:, G * 512 + lo:(G + 1) * 512],
                                start=True,
                                stop=True,
                            )
                        drain_filler(5 if G < 2 else (3 if G == 2 else 2))
                        if idx == 2:
                            for st in pending:
                                emit_attn_tail(st)
                            if dma_G is not None:
                                row_dma(dma_G)
                        flush_pv(keep=1)
                        pt = ptp.tile([128, 1024], BF16, tag="pt",
                                      name=f"pt{hp}_{G}_{idx}")
                        if w == 512:
                            nc.scalar.activation(pt[:], pss[:], AF.Exp,
                                                 scale=SCALE)
                        else:
                            view = "p (b c) -> p b c"
                            nc.scalar.activation(
                                pt[:].rearrange(view, b=2)[:, :, 0:w],
                                pss[:].rearrange(view, b=2)[:, :, 0:w],
                                AF.Exp, scale=SCALE,
                            )
                        if tri:
                            for c0 in (0, 512):
                                nc.vector.tensor_tensor(
                                    pt[:, c0:c0 + 128],
                                    pt[:, c0:c0 + 128],
                                    tri_sb[:], op=mybir.AluOpType.mult,
                                )
                        need_v(cj)

                        def mk_pv(pt=pt, cj=cj, lo=lo, w=w, idx=idx):
                            def emit():
                                psoA, psoB = get_pso()
                                for pso_, h_, c0 in ((psoA, hA, 0),
                                                     (psoB, hB, 512)):
                                    nc.tensor.matmul(
                                        pso_[0:DP, lo:512],
                                        vs[cj][:, h_, :],
                                        pt[:, c0:c0 + w],
                                        start=(idx == 0),
                                        stop=(idx == nseg - 1),
                                    )
                            return emit

                        pv_q.append(mk_pv())
                    psoA, psoB = get_pso()
                    return [(hA, G, psoA), (hB, G, psoB)]

                # ---------- attention stream ----------
                pending, dma_G = [], None
                for G in range(NG):
                    if G > 0:
                        flush_level(G)
                    for hp in range(2):
                        new_tails = emit_attn_pair(hp, G, pending, dma_G)
                        dma_G = G if hp == 1 else None
                        pending = new_tails
                flush_filler()
                flush_pv()
                for st in pending:
                    emit_attn_tail(st)
                for tcc in range(TC - 4, TC):
                    nc.sync.dma_start(
                        out[tcc * 128:(tcc + 1) * 128, :],
                        outs[:, tcc, :],
                    )

    nc.compile()
    return nc


def make_tri():
    """Multiplicative causal mask for a 128x128 diagonal block of S^T[j, i]:
    1 where j <= i (attend), 0 where j > i (future)."""
    jj = np.arange(128)[:, None]
    ii = np.arange(128)[None, :]
    return np.where(jj <= ii, 1.0, 0.0).astype(np.float32)


def core_inputs(x, w_attn, b_attn, core):
    bf16 = mybir.dt.np(BF16)
    b = core // 4
    h0 = (core % 4) * HPC
    q_sl = slice(h0 * D, (h0 + HPC) * D)
    k_sl = slice(C + h0 * D, C + (h0 + HPC) * D)
    v_sl = slice(2 * C + h0 * D, 2 * C + (h0 + HPC) * D)
    return {
        "xT": np.ascontiguousarray(x[b].T).astype(bf16),
        "w_qk": np.ascontiguousarray(
            np.concatenate([w_attn[:, q_sl], w_attn[:, k_sl]], axis=1)
        ).astype(bf16),
        "w_v": np.ascontiguousarray(w_attn[:, v_sl]).astype(bf16),
        "b_qk": np.ascontiguousarray(
            np.concatenate([b_attn[q_sl], b_attn[k_sl]])[:, None],
            dtype=np.float32,
        ),
        "b_v": np.ascontiguousarray(b_attn[v_sl][None, :], dtype=np.float32),
        "tri": make_tri().astype(bf16),
        "ident": np.eye(128, dtype=np.float32).astype(bf16),
    }


_NC_CACHE = None


def run(x, w_attn, b_attn, **spmd_kwargs):
    """Run on the 8 NeuronCores; returns (full_output, BassKernelResults)."""
    global _NC_CACHE
    from concourse.bass_utils import run_bass_kernel_spmd

    x = np.asarray(x, dtype=np.float32)
    w_attn = np.asarray(w_attn, dtype=np.float32)
    b_attn = np.asarray(b_attn, dtype=np.float32)

    if _NC_CACHE is None:
        _NC_CACHE = build_nc()
    nc = _NC_CACHE

    in_maps = [core_inputs(x, w_attn, b_attn, c) for c in range(NCORES)]
    res = run_bass_kernel_spmd(
        nc, in_maps, core_ids=list(range(NCORES)), **spmd_kwargs
    )

    outf = np.empty((B, T, C), dtype=np.float32)
    for c in range(NCORES):
        b = c // 4
        h0 = (c % 4) * HPC
        outf[b, :, h0 * D:(h0 + HPC) * D] = res.results[c]["out"]
    return outf, res


def kernel(x, w_attn, b_attn):
    return run(x, w_attn, b_attn)[0]


# revision 27
# speedup vs baseline: 1.0167x; 1.0167x over previous
"""Multi-head causal self-attention on 8 Trainium2 NeuronCores.

Problem: x[2, 2048, 1024] @ w_attn[1024, 3072] (+b) -> split q,k,v (16 heads,
head_size 64) -> causal softmax attention -> out [2, 2048, 1024].

Sharding: 32 (batch, head) pairs across 8 cores -> each core handles one batch
and 4 consecutive heads (batch = core // 4, heads = (core % 4) * 4 + [0..3]).
No collectives; host assembles the output.

Per-core kernel, v3 (all matmul operands bf16; host pre-casts x/w):
  - q/k projection staged by 512-wide t-groups (tg0 in the prologue, tg1-3
    dribbled as PE filler through the attention stream) -> qkT[n, t] bf16.
  - v projection per 128-row t-chunk, also filler -> V[t, 4, 65] bf16 with a
    ones column (row 64 of the PV output = softmax denominator).
  - attention per (head h, 512-wide query window G): S^T[j, i] at 128-granular
    causal resolution: 4G off-diagonal [128, 512] chunk matmuls plus the four
    diagonal chunks trimmed to their visible suffix (512/384/256/128 wide),
    packed tightly into [128, 1024] PSUM tiles (pieces split at 512-column
    bank boundaries). One wide exp per tile (scale = 1/8; no max-subtraction:
    |logits|*scale < ~8); per-diagonal-chunk [128, 128] tril multiplies on
    the bf16 exp tile (DVE 2x); PV accumulates out^T[d, i] (+denominator row)
    per piece, software-pipelined two PSUM tiles behind QK so the in-order PE
    queue never blocks on an exp.
  - tail per block: PSUM->SBUF copy, 4 PE transposes into the same (just
    freed) PSUM bank, reciprocal of the denominator row, normalize into the
    output tile; outputs DMAd per 512-row group as they complete.
"""

from collections import deque

import numpy as np

import concourse.bacc as bacc
import concourse.tile as tile
from concourse import mybir

AF = mybir.ActivationFunctionType
F32 = mybir.dt.float32
BF16 = mybir.dt.bfloat16

B, T, C = 2, 2048, 1024
H, D = 16, 64
HPC = 4                 # heads per core
NCORES = 8
KC = C // 128           # 8 contraction chunks
TC = T // 128           # 16 key/time chunks of 128
NG = T // 512           # 4 query windows of 512
NQK = 2 * HPC * D       # 512 q+k columns per core
NV = HPC * D            # 256 v columns per core
DP = D + 1              # v columns + ones column
SCALE = D ** -0.5


def build_nc(reps=1):
    """reps > 1 repeats the whole compute serially inside one NEFF —
    used only for differential on-device timing (tunnel overhead cancels)."""
    nc = bacc.Bacc("TRN2", target_bir_lowering=False, debug=False,
                   num_devices=NCORES)

    xT = nc.dram_tensor("xT", [C, T], BF16, kind="ExternalInput")
    w_qk = nc.dram_tensor("w_qk", [C, NQK], BF16, kind="ExternalInput")
    w_v = nc.dram_tensor("w_v", [C, NV], BF16, kind="ExternalInput")
    b_qk = nc.dram_tensor("b_qk", [NQK, 1], F32, kind="ExternalInput")
    b_v = nc.dram_tensor("b_v", [1, NV], F32, kind="ExternalInput")
    tri = nc.dram_tensor("tri", [128, 128], BF16, kind="ExternalInput")
    ident = nc.dram_tensor("ident", [128, 128], BF16, kind="ExternalInput")
    out = nc.dram_tensor("out", [T, NV], F32, kind="ExternalOutput")

    with tile.TileContext(nc) as tc:
      for rep in range(reps):
        with (
            tc.tile_pool(name=f"const{rep}", bufs=1) as cpool,
            tc.tile_pool(name=f"xw{rep}", bufs=1) as xw,
            tc.tile_pool(name=f"qkv{rep}", bufs=1) as qkv,
            tc.tile_pool(name=f"outp{rep}", bufs=1) as outp,
            tc.tile_pool(name=f"ptp{rep}", bufs=4) as ptp,
            tc.tile_pool(name=f"otp{rep}", bufs=2) as otp,
            tc.tile_pool(name=f"recp{rep}", bufs=2) as recp,
            tc.tile_pool(name=f"warmp{rep}", bufs=1) as warmp,
        ):
            # ---------- ACT exp-table warmup (hide the first-use load) ------
            warm = warmp.tile([128, 1], F32, tag="warm")
            nc.vector.memset(warm[:], 0.0)
            nc.scalar.activation(warm[:], warm[:], AF.Exp)

            # ---------- DMAs, in consumption order, batched ----------
            wqk_sb = xw.tile([128, KC, NQK], BF16)
            wv_sb = xw.tile([128, KC, NV], BF16)
            xth = [xw.tile([128, KC, 512], BF16, tag=f"xt{tg}",
                           name=f"xt{tg}") for tg in range(NG)]
            wqk_r = w_qk[:].rearrange("(kc p) n -> p kc n", p=128)
            xT_r = xT[:].rearrange("(kc p) t -> p kc t", p=128)
            first_groups = [slice(0, 2), slice(2, 4), slice(4, 6),
                            slice(6, 8)]
            for kcs in first_groups:
                nc.sync.dma_start(wqk_sb[:, kcs, :], wqk_r[:, kcs, :])
                nc.sync.dma_start(xth[0][:, kcs, :], xT_r[:, kcs, 0:512])
            bqk_sb = cpool.tile([128, 4, 1], F32)
            nc.sync.dma_start(
                bqk_sb[:], b_qk[:].rearrange("(c p) one -> p c one", p=128)
            )
            nc.sync.dma_start(
                wv_sb[:], w_v[:].rearrange("(kc p) n -> p kc n", p=128)
            )
            bv_sb = cpool.tile([128, NV], F32)
            nc.sync.dma_start(bv_sb[:], b_v[:].to_broadcast([128, NV]))
            tri_sb = cpool.tile([128, 128], BF16)
            nc.sync.dma_start(tri_sb[:], tri[:])
            ident_sb = cpool.tile([128, 128], BF16)
            nc.sync.dma_start(ident_sb[:], ident[:])
            for tg in range(1, NG):
                nc.sync.dma_start(
                    xth[tg][:], xT_r[:, :, tg * 512:(tg + 1) * 512]
                )

            # ---------- persistent sbuf tiles ----------
            qkts = {n: qkv.tile([128, T], BF16, tag=f"qk{n}", name=f"qk{n}")
                    for n in range(4)}
            vs = [None] * TC
            outs = outp.tile([128, TC, NV], F32, tag="outs", name="outs")

            with (
                tc.tile_pool(name=f"fill{rep}", bufs=1, space="PSUM") as fillp,
                tc.tile_pool(name=f"psS{rep}", bufs=2, space="PSUM") as psS,
                tc.tile_pool(name=f"pso{rep}", bufs=3, space="PSUM") as psop,
            ):
                filler = deque()   # (level, emit_fn) items, FIFO
                pv_q = deque()     # deferred PV emitters, depth 2

                def emit_proj_group(tg, n, pool=None, tag="fill"):
                    """qkT[n-chunk, tg-group] = w_qk[:,nchunk].T @ x^T + b."""
                    pp = (pool or fillp).tile([128, 512], F32, tag=tag,
                                              name=f"pp{tg}_{n}")

                    def mk_mm(kc):
                        def emit():
                            nc.tensor.matmul(
                                pp[:],
                                wqk_sb[:, kc, n * 128:(n + 1) * 128],
                                xth[tg][:, kc, :],
                                start=(kc == 0),
                                stop=(kc == KC - 1),
                            )
                        return emit

                    def fini():
                        nc.vector.tensor_scalar_add(
                            qkts[n][:, tg * 512:(tg + 1) * 512],
                            pp[:], bqk_sb[:, n, :],
                        )

                    return [mk_mm(kc) for kc in range(KC)] + [fini]

                def emit_vproj_group(jc, pool=None, tag="fill"):
                    """vs[jc] = x^T[:, jc].T @ w_v + b_v, plus ones column."""
                    pv = (pool or fillp).tile([128, 512], F32, tag=tag,
                                              name=f"pv{jc}")
                    vt = qkv.tile([128, HPC, DP], BF16, tag=f"v{jc}",
                                  name=f"v{jc}")

                    def mk_mm(kc):
                        def emit():
                            nc.tensor.matmul(
                                pv[:, 0:NV],
                                xth[jc // 4][:, kc, (jc % 4) * 128:
                                             (jc % 4 + 1) * 128],
                                wv_sb[:, kc, :],
                                start=(kc == 0),
                                stop=(kc == KC - 1),
                            )
                        return emit

                    def fini():
                        nc.vector.tensor_tensor(
                            vt[:, :, 0:D],
                            pv[:, 0:NV].rearrange("p (h d) -> p h d", d=D),
                            bv_sb[:].rearrange("p (h d) -> p h d", d=D),
                            op=mybir.AluOpType.add,
                        )
                        nc.vector.memset(vt[:, :, D:DP], 1.0)
                        vs[jc] = vt

                    return [mk_mm(kc) for kc in range(KC)] + [fini]

                def drain_filler(n):
                    for _ in range(n):
                        if filler:
                            filler.popleft()[1]()

                def flush_level(level):
                    while filler and filler[0][0] <= level:
                        filler.popleft()[1]()

                def flush_filler():
                    while filler:
                        filler.popleft()[1]()

                def need_v(cj):
                    while vs[cj] is None:
                        assert filler, f"v{cj} not queued"
                        filler.popleft()[1]()

                def flush_pv(keep=0):
                    while len(pv_q) > keep:
                        pv_q.popleft()()

                # ---------- prologue: tg0 projection + v chunks 0..3 ----
                # borrow the (still idle) psS pool so consecutive groups
                # rotate over two banks instead of serializing on one
                for n in range(4):
                    for it in emit_proj_group(0, n, pool=psS, tag="psS"):
                        it()
                for jc in range(4):
                    for it in emit_vproj_group(jc, pool=psS, tag="psS"):
                        it()

                # ---------- filler queue: tg1..3 proj + v chunks 4..15 ----
                for lvl in (1, 2, 3):
                    for n in range(4):
                        for it in emit_proj_group(lvl, n):
                            filler.append((lvl, it))
                    for jc in range(4 * lvl, 4 * lvl + 4):
                        for it in emit_vproj_group(jc):
                            filler.append((lvl + 0.6, it))

                def emit_attn_tail(state, final_dma=False):
                    """Transpose out^T, normalize by the denominator row.
                    Transposes reuse the pso PSUM bank just read. On the
                    final block, each output t-chunk DMAs out right after
                    its normalize so HWDGE overlaps the DVE tail."""
                    h, G, pso = state
                    oT = otp.tile([DP, 512], BF16, tag="oT",
                                  name=f"oT{h}_{G}")
                    nc.vector.tensor_copy(oT[:], pso[0:DP, :])
                    # 66-wide bf16 slots keep each transpose output 4B-aligned
                    pt2 = pso[:, 0:132].bitcast(BF16).rearrange(
                        "p (ic d) -> p ic d", d=DP + 1
                    )
                    for ic in range(4):
                        nc.tensor.matmul(
                            pt2[:, ic, 0:DP],
                            oT[:, ic * 128:(ic + 1) * 128],
                            ident_sb[0:DP, 0:DP],
                            is_transpose=True,
                            start=(ic == 0),
                            stop=(ic == 3),
                        )
                    rec = recp.tile([128, 4, 1], F32, tag="rec",
                                    name=f"rec{h}_{G}")
                    nc.vector.reciprocal(rec[:], pt2[:, :, D:DP])
                    for ic in range(4):
                        nc.vector.tensor_scalar_mul(
                            outs[:, 4 * G + ic, h * D:(h + 1) * D],
                            pt2[:, ic, 0:D], rec[:, ic, :],
                        )
                        if final_dma:
                            tcc = 4 * G + ic
                            nc.sync.dma_start(
                                out[tcc * 128:(tcc + 1) * 128, :],
                                outs[:, tcc, :],
                            )

                def row_dma(G):
                    nc.sync.dma_start(
                        out[G * 512:(G + 1) * 512, :]
                        .rearrange("(tc p) n -> p tc n", p=128),
                        outs[:, 4 * G:4 * G + 4, :],
                    )

                def emit_attn_pair(hp, G, pending, dma_G):
                    """Paired attention block: heads A=2hp (PE rows 0:64)
                    and B=2hp+1 (rows 64:128). Each pack tile holds one
                    causal piece for BOTH heads ([A @ bank0 | B @ bank1]):
                    the two QK matmuls have disjoint row-groups
                    (tile_position auto-derives from base_partition) and
                    disjoint PSUM banks, so the PE runs them concurrently.
                    `pending` tails (previous pair) are emitted after pack
                    2, before this pair's pso allocations claim their
                    banks; `dma_G` row group is flushed right after."""
                    hA, hB = 2 * hp, 2 * hp + 1
                    qTA = qkts[hp][0:D, :]
                    qTB = qkts[hp][D:128, :]
                    kTA = qkts[2 + hp][0:D, :]
                    kTB = qkts[2 + hp][D:128, :]
                    segs = [(cj, 0, False) for cj in range(4 * G)]
                    segs += [(4 * G + v, 128 * v, True) for v in range(4)]
                    nseg = len(segs)
                    hold = {}

                    def get_pso():
                        if not hold:
                            hold["A"] = psop.tile(
                                [128, 512], F32, tag="pso",
                                name=f"psoA{hp}_{G}")
                            hold["B"] = psop.tile(
                                [128, 512], F32, tag="pso",
                                name=f"psoB{hp}_{G}")
                        return hold["A"], hold["B"]

                    for idx, (cj, lo, tri) in enumerate(segs):
                        w = 512 - lo
                        pss = psS.tile([128, 1024], F32, tag="psS",
                                       name=f"pss{hp}_{G}_{idx}")
                        for kT, qT, c0 in ((kTA, qTA, 0), (kTB, qTB, 512)):
                            nc.tensor.matmul(
                                pss[:, c0:c0 + w],
                                kT[:, cj * 128:(cj + 1) * 128],
                                qT[:, G * 512 + lo:(G + 1) * 512],
                                start=True,
                                stop=True,
                            )
                        drain_filler(5 if G < 2 else (3 if G == 2 else 2))
                        if idx == 2:
                            for st in pending:
                                emit_attn_tail(st)
                            if dma_G is not None:
                                row_dma(dma_G)
                        flush_pv(keep=1)
                        pt = ptp.tile([128, 1024], BF16, tag="pt",
                                      name=f"pt{hp}_{G}_{idx}")
                        if w == 512:
                            nc.scalar.activation(pt[:], pss[:], AF.Exp,
                                                 scale=SCALE)
                        else:
                            view = "p (b c) -> p b c"
                            nc.scalar.activation(
                                pt[:].rearrange(view, b=2)[:, :, 0:w],
                                pss[:].rearrange(view, b=2)[:, :, 0:w],
                                AF.Exp, scale=SCALE,
                            )
                        if tri:
                            for c0 in (0, 512):
                                nc.vector.tensor_tensor(
                                    pt[:, c0:c0 + 128],
                                    pt[:, c0:c0 + 128],
                                    tri_sb[:], op=mybir.AluOpType.mult,
                                )
                        need_v(cj)

                        def mk_pv(pt=pt, cj=cj, lo=lo, w=w, idx=idx):
                            def emit():
                                psoA, psoB = get_pso()
                                for pso_, h_, c0 in ((psoA, hA, 0),
                                                     (psoB, hB, 512)):
                                    nc.tensor.matmul(
                                        pso_[0:DP, lo:512],
                                        vs[cj][:, h_, :],
                                        pt[:, c0:c0 + w],
                                        start=(idx == 0),
                                        stop=(idx == nseg - 1),
                                    )
                            return emit

                        pv_q.append(mk_pv())
                    psoA, psoB = get_pso()
                    return [(hA, G, psoA), (hB, G, psoB)]

                # ---------- attention stream ----------
                pending, dma_G = [], None
                for G in range(NG):
                    if G > 0:
                        flush_level(G)
                    for hp in range(2):
                        new_tails = emit_attn_pair(hp, G, pending, dma_G)
                        dma_G = G if hp == 1 else None
                        pending = new_tails
                flush_filler()
                flush_pv()
                for st in pending:
                    emit_attn_tail(st, final_dma=(st is pending[-1]))

    nc.compile()
    return nc


def make_tri():
    """Multiplicative causal mask for a 128x128 diagonal block of S^T[j, i]:
    1 where j <= i (attend), 0 where j > i (future)."""
    jj = np.arange(128)[:, None]
    ii = np.arange(128)[None, :]
    return np.where(jj <= ii, 1.0, 0.0).astype(np.float32)


def core_inputs(x, w_attn, b_attn, core):
    bf16 = mybir.dt.np(BF16)
    b = core // 4
    h0 = (core % 4) * HPC
    q_sl = slice(h0 * D, (h0 + HPC) * D)
    k_sl = slice(C + h0 * D, C + (h0 + HPC) * D)
    v_sl = slice(2 * C + h0 * D, 2 * C + (h0 + HPC) * D)
    return {
        "xT": np.ascontiguousarray(x[b].T).astype(bf16),
        "w_qk": np.ascontiguousarray(
            np.concatenate([w_attn[:, q_sl], w_attn[:, k_sl]], axis=1)
        ).astype(bf16),
        "w_v": np.ascontiguousarray(w_attn[:, v_sl]).astype(bf16),
        "b_qk": np.ascontiguousarray(
            np.concatenate([b_attn[q_sl], b_attn[k_sl]])[:, None],
            dtype=np.float32,
        ),
        "b_v": np.ascontiguousarray(b_attn[v_sl][None, :], dtype=np.float32),
        "tri": make_tri().astype(bf16),
        "ident": np.eye(128, dtype=np.float32).astype(bf16),
    }


_NC_CACHE = None


def run(x, w_attn, b_attn, **spmd_kwargs):
    """Run on the 8 NeuronCores; returns (full_output, BassKernelResults)."""
    global _NC_CACHE
    from concourse.bass_utils import run_bass_kernel_spmd

    x = np.asarray(x, dtype=np.float32)
    w_attn = np.asarray(w_attn, dtype=np.float32)
    b_attn = np.asarray(b_attn, dtype=np.float32)

    if _NC_CACHE is None:
        _NC_CACHE = build_nc()
    nc = _NC_CACHE

    in_maps = [core_inputs(x, w_attn, b_attn, c) for c in range(NCORES)]
    res = run_bass_kernel_spmd(
        nc, in_maps, core_ids=list(range(NCORES)), **spmd_kwargs
    )

    outf = np.empty((B, T, C), dtype=np.float32)
    for c in range(NCORES):
        b = c // 4
        h0 = (c % 4) * HPC
        outf[b, :, h0 * D:(h0 + HPC) * D] = res.results[c]["out"]
    return outf, res


def kernel(x, w_attn, b_attn):
    return run(x, w_attn, b_attn)[0]
